# revision 23
# baseline (speedup 1.0000x reference)
"""Self-contained TRN2 Bass kernel for nn_Attention (B=4, N=2048, D=1024, H=16).

Sharding: 8 NeuronCores, core c = (batch b = c//2, head-half = c%2).
Each core computes causal attention for its batch and 8 of 16 heads plus the
row-parallel half of the output projection; the host sums the two half-partials
per batch.

Per-core pipeline (all on-device, Tile-scheduled):
  xT [D, NT] f32  --qk-proj (bf16 matmuls)-->  qT/kT [128 feats, NT] bf16
                  --v-proj--->                 V [token, feat] bf16 (+ones col)
  S^T chunk [128 k-tok, 512 q-tok] = kT-part @ qT   (bf16, f32 psum)
  exp on ScalarE (pure Exp — no act-table thrash); causal triangle mask
  applied post-exp as a 0/1 multiply on DVE (keeps the PE free of mask work)
  O^T + softmax denominator via P^T @ [V | 1]
  den reciprocal on DVE; partition-broadcast + normalize multiply on GpSimd
  partial out [NT, E] = OT.T @ woutT  (bf16 matmuls, f32 psum, bf16 store)
"""

import os
import sys
import types
from contextlib import ExitStack
from dataclasses import dataclass

for _p in ('/opt/trn_rl_repo', '/root/.axon_site/_ro/trn_rl_repo'):
    if os.path.isdir(_p) and _p not in sys.path:
        sys.path.append(_p)

import numpy as np
import ml_dtypes

import concourse.bass as bass
import concourse.mybir as mybir
import concourse.tile as tile
from concourse import bacc

F32 = mybir.dt.float32
BF16 = mybir.dt.bfloat16


# ---------------------------------------------------------------- harness fixes
def _install_ntff_hook():
    """Register the axon NTFF profile hook that trn_boot skips when the
    container's antenv stub lacks axon_hooks (needed only for trace=True)."""
    if 'antenv.axon_hooks' in sys.modules:
        return
    try:
        import antenv
        mod = types.ModuleType('antenv.axon_hooks')
        _hook = [None]
        mod.set_axon_ntff_profile_hook = lambda h: _hook.__setitem__(0, h)
        mod.get_axon_ntff_profile_hook = lambda: _hook[0]
        sys.modules['antenv.axon_hooks'] = mod
        antenv.axon_hooks = mod
        from trn_agent_boot.trn_boot import _ntff_profile_via_ctypes
        so = '/opt/axon/libaxon_pjrt.so'
        if os.path.exists(so):
            hook = _ntff_profile_via_ctypes(so)
            if hook is not None:
                mod.set_axon_ntff_profile_hook(hook)
    except Exception:
        pass


def _patch_tile_drain():
    """walrus TPB_CTRL encodes <=2 sync waits; Tile's tail drain can carry
    more. Split extras onto single-wait nops (sequentially equivalent)."""
    import concourse.tile as tile_mod
    if getattr(tile_mod.TileContext, '_drain_patched', False):
        return
    from concourse.tile import ScopedClock

    def _drain_and_barrier(self, tick_clock, wait_clock):
        nc = self.nc
        drain_inst = nc.sync.drain()
        wait_clock.add_sem_waits(
            drain_inst.ins, ScopedClock({None: tick_clock.global_clock}))
        si = drain_inst.ins.sync_info
        if si is not None and si.on_wait and len(si.on_wait) > 1:
            waits = list(si.on_wait)
            drain_inst.ins.sync_info = mybir.SyncInfo(
                on_wait=waits[:1], on_update=list(si.on_update or []))
            for w in waits[1:]:
                nop = nc.sync.nop(nofuse=True)
                nop.ins.sync_info = mybir.SyncInfo(on_wait=[w], on_update=[])
        nc.all_engine_barrier()
        popped = nc._tile_sem_poison_stack.pop()
        assert popped is self._sem_poison
        nc.clear_and_free_semaphores(list(self.sems.allocated().values()))
        nc.all_engine_barrier()

    tile_mod.TileContext._drain_and_barrier = _drain_and_barrier
    tile_mod.TileContext._drain_patched = True


# ---------------------------------------------------------------- kernel build
@dataclass(frozen=True)
class Cfg:
    NT: int = 2048   # tokens
    D: int = 1024    # model dim
    HH: int = 8      # heads per core
    DH: int = 64     # head dim
    E: int = 1024    # output features
    QC: int = 512    # q-chunk (free dim of score tiles)
    KC: int = 128    # k-tile (partition dim of score tiles)
    PT_BUFS: int = 8
    PSS_BUFS: int = 2
    PSO_BUFS: int = 2
    OCP_BUFS: int = 6
    DEN_BUFS: int = 6
    PSA_BUFS: int = 4

    @property
    def DC(self): return self.D // 128
    @property
    def NKT(self): return self.NT // self.KC
    @property
    def NQC(self): return self.NT // self.QC
    @property
    def NPAIR(self): return self.HH // 2
    @property
    def QF(self): return self.HH * self.DH
    @property
    def VW(self): return self.DH + 1


def build(cfg: Cfg) -> bass.Bass:
    _patch_tile_drain()
    nc = bacc.Bacc('TRN2', target_bir_lowering=False)
    c = cfg
    assert c.QC % c.KC == 0 and c.NT % c.QC == 0 and c.D % 128 == 0
    assert c.DH * 2 == c.KC
    JPT = c.QC // c.KC
    NTT = c.QC // 128

    xT = nc.declare_dram_parameter("xT", [c.DC, 128, c.NT], BF16, isOutput=False)
    wqk = nc.declare_dram_parameter("wqk", [c.DC, 128, 2 * c.QF], BF16, isOutput=False)
    wv = nc.declare_dram_parameter("wv", [c.DC, 128, c.QF], BF16, isOutput=False)
    wout = nc.declare_dram_parameter("wout", [128, c.NPAIR, c.E], BF16, isOutput=False)
    tri = nc.declare_dram_parameter("tri", [128, c.KC], BF16, isOutput=False)
    out = nc.declare_dram_parameter("out", [c.NT, c.E], BF16, isOutput=True)

    with tile.TileContext(nc) as tc, ExitStack() as ctx:
        const = ctx.enter_context(tc.tile_pool(name="const", bufs=1))
        persist = ctx.enter_context(tc.tile_pool(name="persist", bufs=1))

        # ---------------- startup-ordered DMAs ----------------
        # critical prefix: tri + wv + xT quarter 0 (feeds phase A), then wqk
        # (feeds qk prologue), then xT q1, wout, xT q2-3.
        tri_sb = const.tile([128, c.KC], BF16)
        nc.sync.dma_start(out=tri_sb[:], in_=tri[:])
        ones_bf = const.tile([128, c.DH], BF16)
        nc.vector.memset(ones_bf[:], 1.0)
        wv_sb = const.tile([128, c.DC, c.QF], BF16)
        for dc in range(c.DC):
            nc.sync.dma_start(out=wv_sb[:, dc, :], in_=wv[dc])

        xt_t = [persist.tile([128, c.NT], BF16, tag=f"xt{dc}", name=f"xt{dc}")
                for dc in range(c.DC)]
        q0 = slice(0, c.QC)
        for dc in range(c.DC):
            nc.gpsimd.dma_start(out=xt_t[dc][:, q0], in_=xT[dc][:, q0])

        wqk_sb = const.tile([128, c.DC, 2 * c.QF], BF16)
        for dc in range(c.DC):
            nc.sync.dma_start(out=wqk_sb[:, dc, :], in_=wqk[dc])

        q1 = slice(c.QC, 2 * c.QC)
        for dc in range(c.DC):
            nc.gpsimd.dma_start(out=xt_t[dc][:, q1], in_=xT[dc][:, q1])

        wout_sb = const.tile([128, c.NPAIR, c.E], BF16)
        nc.sync.dma_start(out=wout_sb[:], in_=wout[:])

        qrest = slice(2 * c.QC, c.NT)
        for dc in range(c.DC):
            nc.gpsimd.dma_start(out=xt_t[dc][:, qrest], in_=xT[dc][:, qrest])

        qk_sb = [persist.tile([128, c.NT], BF16, tag=f"qk{e}", name=f"qk{e}")
                 for e in range(2 * c.NPAIR)]
        V_sb = persist.tile([128, c.NKT, c.HH, c.VW], BF16, tag="V", name="V_sb")
        nc.vector.memset(V_sb[:, :, :, c.DH], 1.0)
        OT_sb = [persist.tile([128, c.NT], BF16, tag=f"ot{p}", name=f"ot{p}")
                 for p in range(c.NPAIR)]

        # ---------------- Phase B: attention ----------------
        with (
            tc.tile_pool(name="pt", bufs=c.PT_BUFS) as pt_pool,
            tc.tile_pool(name="ocp", bufs=c.OCP_BUFS) as ocp_pool,
            tc.tile_pool(name="den", bufs=c.DEN_BUFS) as den_pool,
            tc.tile_pool(name="otst", bufs=4) as otst_pool,
            tc.tile_pool(name="dend", bufs=4, space="DRAM") as dend_pool,
            tc.tile_pool(name="ps_s", bufs=c.PSS_BUFS, space="PSUM") as ps_s,
            tc.tile_pool(name="ps_o", bufs=c.PSO_BUFS, space="PSUM") as ps_o,
            tc.tile_pool(name="ps_f", bufs=2, space="PSUM") as ps_f,
            tc.tile_pool(name="osbB", bufs=3) as outB_pool,
        ):
            def evac_stage1(psO):
                # free the psum banks fast: one copy per head (O rows + den)
                ocps = []
                for h2 in range(2):
                    ocp = ocp_pool.tile([c.VW, c.QC], F32, tag="ocp",
                                        name="ocp")
                    nc.vector.tensor_copy(out=ocp[:], in_=psO[h2][:])
                    ocps.append(ocp)
                return ocps

            def evac_stage2(p, t, ocps):
                # latency-tolerant: den row -> DRAM bounce partition
                # broadcast -> fast DVE reciprocal (needs partition base 0)
                # -> normalize multiply on the otherwise-idle Pool engine
                qsl_full = slice(t * c.QC, (t + 1) * c.QC)
                for h2 in range(2):
                    ocp = ocps[h2]
                    den_d = dend_pool.tile([c.QC], F32, tag="dend",
                                           name="den_d")
                    nc.sync.dma_start(out=den_d[:], in_=ocp[c.DH:c.VW, :])
                    divB = den_pool.tile([c.DH, c.QC], F32, tag="div",
                                         name="divB", bufs=4)
                    nc.sync.dma_start(
                        out=divB[:], in_=den_d.partition_broadcast(c.DH))
                    rdiv = den_pool.tile([c.DH, c.QC], F32, tag="rdiv",
                                         name="rdiv", bufs=4)
                    nc.vector.reciprocal_approx_fast(
                        out=rdiv[:], in_=divB[:])
                    if h2 == 0:
                        nc.gpsimd.tensor_tensor(
                            out=OT_sb[p][0:c.DH, qsl_full],
                            in0=ocp[0:c.DH, :],
                            in1=rdiv[:], op=mybir.AluOpType.mult,
                        )
                    else:
                        ot_st = otst_pool.tile([c.DH, c.QC], BF16, tag="ot",
                                               name="ot_st")
                        nc.gpsimd.tensor_tensor(
                            out=ot_st[:], in0=ocp[0:c.DH, :], in1=rdiv[:],
                            op=mybir.AluOpType.mult,
                        )
                        nc.sync.dma_start(
                            out=OT_sb[p][c.DH:2 * c.DH, qsl_full],
                            in_=ot_st[:])

            def emit_proj_chunk(nt, ec, evac=None):
                esl = slice(ec * c.QC, (ec + 1) * c.QC)
                psP = ps_f.tile([128, c.QC], F32, tag="f", name="psP")
                for pr in range(c.NPAIR):
                    nc.tensor.matmul(
                        psP[:],
                        lhsT=OT_sb[pr][:, nt * 128:(nt + 1) * 128],
                        rhs=wout_sb[:, pr, esl],
                        start=(pr == 0), stop=(pr == c.NPAIR - 1),
                    )
                o_sb = outB_pool.tile([128, c.QC], BF16, tag="ob", name="o_sb")
                if evac == 'scalar':
                    nc.scalar.copy(out=o_sb[:], in_=psP[:])
                else:
                    nc.vector.tensor_copy(out=o_sb[:], in_=psP[:])
                nc.sync.dma_start(
                    out=out[nt * 128:(nt + 1) * 128, esl], in_=o_sb[:])

            def emit_qk_chunk(pp, ci):
                e = (pp, c.NPAIR + pp)[ci // c.NQC]
                ncc = ci % c.NQC
                nsl = slice(ncc * c.QC, (ncc + 1) * c.QC)
                psqk = ps_f.tile([128, c.QC], F32, tag="f", name="psqk")
                for dc in range(c.DC):
                    nc.tensor.matmul(
                        psqk[:],
                        lhsT=wqk_sb[:, dc, e * 128:(e + 1) * 128],
                        rhs=xt_t[dc][:, nsl],
                        start=(dc == 0), stop=(dc == c.DC - 1),
                    )
                nc.vector.tensor_copy(out=qk_sb[e][:, nsl], in_=psqk[:])

            def emit_v_chunk(nt):
                psv = ps_f.tile([128, c.QF], F32, tag="f", name="psvf")
                for dc in range(c.DC):
                    nc.tensor.matmul(
                        psv[:],
                        lhsT=xt_t[dc][:, nt * 128:(nt + 1) * 128],
                        rhs=wv_sb[:, dc, :],
                        start=(dc == 0), stop=(dc == c.DC - 1),
                    )
                nc.scalar.copy(
                    out=V_sb[:, nt, :, 0:c.DH],
                    in_=psv[:].rearrange("p (h f) -> p h f", h=c.HH),
                )

            pending2 = None
            # prologue: first q-chunk's V rows, then its q and k columns
            for nt in range(min(NTT, c.NKT)):
                emit_v_chunk(nt)
            emit_qk_chunk(0, 0)
            emit_qk_chunk(0, c.NQC)
            for p in range(c.NPAIR):
                q_t, k_t = qk_sb[p], qk_sb[c.NPAIR + p]
                last_pair = (p == c.NPAIR - 1)
                for t in range(c.NQC):
                    njt = JPT * t + JPT
                    psO = [ps_o.tile([c.VW, c.QC], F32, tag="o", name=f"psO{_h}")
                           for _h in range(2)]
                    def emit_pv(items):
                        for (h2_, pt_, lo_, j_) in items:
                            nc.tensor.matmul(
                                psO[h2_][:, lo_:],
                                lhsT=V_sb[:, j_, 2 * p + h2_, :],
                                rhs=pt_[:, h2_, lo_:],
                                start=(j_ == 0), stop=(j_ == njt - 1),
                            )

                    # filler work (future qk/v projections, or output-
                    # projection chunks during the last pair), interleaved
                    # INSIDE the j-loop: the attention stream alone is paced
                    # by ScalarE's exps, so the PE needs independent matmuls
                    # queued between PV steps to stay busy
                    if p == 0:
                        fillers = []
                        if t + 1 < c.NQC:
                            # next q-chunk's V rows and q/k columns
                            fillers += [
                                lambda nt_=nt_: emit_v_chunk(nt_)
                                for nt_ in range((t + 1) * NTT,
                                                 min((t + 2) * NTT, c.NKT))]
                            fillers += [
                                lambda ci=ci: emit_qk_chunk(0, ci)
                                for ci in (t + 1, c.NQC + t + 1)]
                        fillers += [lambda i=i: emit_qk_chunk(1, 2 * t + i)
                                    for i in range(2)]
                    elif not last_pair:
                        fillers = [lambda i=i: emit_qk_chunk(p + 1, 2 * t + i)
                                   for i in range(2)]
                    elif t >= 1:
                        tp = t - 1
                        fillers = [
                            lambda nt_=nt_, ec_=ec_: emit_proj_chunk(nt_, ec_)
                            for nt_ in range(tp * NTT, (tp + 1) * NTT)
                            for ec_ in range(c.E // c.QC)]
                    else:
                        fillers = []
                    # last pair's proj fillers need OT[p] from the previous
                    # chunk's stage2 Pool chain (~4us) — hold them back a few
                    # j-steps; other fillers can start almost immediately
                    fill_start = 6 if last_pair else 1
                    spread = {}
                    if fillers and njt > fill_start:
                        span = njt - fill_start
                        nf = len(fillers)
                        for i, f in enumerate(fillers):
                            jj = fill_start + (i * span) // nf
                            spread.setdefault(jj, []).append(f)
                        fillers = []

                    pipe = []
                    for j in range(njt):
                        off = j * c.KC - t * c.QC
                        band = off >= 0
                        lo = max(off, 0)
                        jsl = slice(j * c.KC, (j + 1) * c.KC)
                        qsl = slice(t * c.QC + lo, (t + 1) * c.QC)
                        # both heads' scores into one 2-bank psum tile
                        psS = ps_s.tile([128, 2, c.QC], F32, tag="s", name="psS")
                        for h2 in range(2):
                            hsl = slice(h2 * c.DH, (h2 + 1) * c.DH)
                            nc.tensor.matmul(
                                psS[:, h2, lo:], lhsT=k_t[hsl, jsl],
                                rhs=q_t[hsl, qsl], start=True, stop=True,
                            )
                        pt_t = pt_pool.tile([128, 2, c.QC], BF16, tag="pt",
                                            name="pt_t")
                        nc.scalar.activation(
                            out=pt_t[:, :, lo:], in_=psS[:, :, lo:],
                            func=mybir.ActivationFunctionType.Exp,
                        )
                        if band:
                            # causal triangle: zero the upper-left of the
                            # diagonal block post-exp (0/1 multiply on DVE)
                            for h2 in range(2):
                                nc.vector.tensor_tensor(
                                    out=pt_t[:, h2, off:off + c.KC],
                                    in0=pt_t[:, h2, off:off + c.KC],
                                    in1=tri_sb[:],
                                    op=mybir.AluOpType.mult,
                                )
                        pipe.append([(0, pt_t, lo, j), (1, pt_t, lo, j)])
                        if len(pipe) > 4:
                            emit_pv(pipe.pop(0))
                        for f in spread.pop(j, []):
                            f()
                    # drain the PV pipe (interleaved with any leftover
                    # fillers), then evacuate psO IMMEDIATELY — before the
                    # remaining fillers' DVE evacs — so the psum banks
                    # recycle in time for the next chunk's first PV
                    while pipe:
                        if fillers:
                            fillers.pop(0)()
                        emit_pv(pipe.pop(0))
                    stage1_done = False
                    if not (last_pair and t == c.NQC - 1):
                        ocps = evac_stage1(psO)
                        stage1_done = True
                    while fillers:
                        fillers.pop(0)()
                    if last_pair and t == c.NQC - 1:
                        # fast tail: den partition-broadcast via a K=1 PE
                        # matmul (no DRAM bounce latency), recip + normalize
                        # on DVE straight from psum, then the final projs
                        qsl_full = slice(t * c.QC, (t + 1) * c.QC)
                        den_bf = den_pool.tile([128, 2, c.QC], BF16,
                                               tag="denb", name="den_bf")
                        for h2 in range(2):
                            nc.vector.tensor_copy(
                                out=den_bf[c.DH:c.VW, h2, :],
                                in_=psO[h2][c.DH:c.VW, :])
                        rdivs = []
                        for h2 in range(2):
                            psB = ps_f.tile([c.DH, c.QC], F32, tag="f",
                                            name="psB")
                            nc.tensor.matmul(
                                psB[:], lhsT=ones_bf[c.DH:c.VW, :],
                                rhs=den_bf[c.DH:c.VW, h2, :],
                                start=True, stop=True,
                            )
                            rdiv = den_pool.tile([c.DH, c.QC], F32,
                                                 tag="rdiv", name="rdiv",
                                                 bufs=4)
                            nc.vector.reciprocal_approx_fast(
                                out=rdiv[:], in_=psB[:])
                            rdivs.append(rdiv)
                        ot_st = otst_pool.tile([c.DH, c.QC], BF16, tag="ot",
                                               name="ot_st")
                        nc.vector.tensor_tensor(
                            out=ot_st[:], in0=psO[1][0:c.DH, :],
                            in1=rdivs[1][:], op=mybir.AluOpType.mult,
                        )
                        nc.sync.dma_start(
                            out=OT_sb[p][c.DH:2 * c.DH, qsl_full],
                            in_=ot_st[:])
                        nc.vector.tensor_tensor(
                            out=OT_sb[p][0:c.DH, qsl_full],
                            in0=psO[0][0:c.DH, :],
                            in1=rdivs[0][:], op=mybir.AluOpType.mult,
                        )
                        # final q-chunk projection (evac on idle ACT)
                        for nt_ in range(t * NTT, (t + 1) * NTT):
                            for ec_ in range(c.E // c.QC):
                                emit_proj_chunk(nt_, ec_, evac='scalar')
                    else:
                        assert stage1_done
                        if pending2 is not None:
                            evac_stage2(*pending2)
                            pending2 = None
                        if last_pair:
                            # immediate: next chunk's proj needs OT[p=3]
                            evac_stage2(p, t, ocps)
                        else:
                            pending2 = (p, t, ocps)

    nc.compile()
    return nc


# ---------------------------------------------------------------- host side
def make_core_inputs(xb, w_qkv, w_out, mask, cfg, half):
    c = cfg
    D = c.D
    scale = 1.0 / np.sqrt(c.DH)
    heads = range(half * c.HH, (half + 1) * c.HH)
    q_rows = np.concatenate(
        [w_qkv[h * c.DH:(h + 1) * c.DH, :] for h in heads]) * scale
    k_rows = np.concatenate(
        [w_qkv[D + h * c.DH:D + (h + 1) * c.DH, :] for h in heads])
    v_rows = np.concatenate(
        [w_qkv[2 * D + h * c.DH:2 * D + (h + 1) * c.DH, :] for h in heads])
    wqk_t = np.concatenate([q_rows, k_rows], axis=0).T  # [D, 2QF]
    wqk_t = np.ascontiguousarray(
        wqk_t.reshape(c.DC, 128, 2 * c.QF)).astype(ml_dtypes.bfloat16)
    wv_t = np.ascontiguousarray(
        v_rows.T.reshape(c.DC, 128, c.QF)).astype(ml_dtypes.bfloat16)
    wo = w_out[:, half * c.QF:(half + 1) * c.QF].T  # [QF, E]
    wo = np.ascontiguousarray(
        wo.reshape(c.NPAIR, 128, c.E).transpose(1, 0, 2)).astype(
            ml_dtypes.bfloat16)
    # 0/1 causal keep-mask for the diagonal block, [k-local, q-local]:
    # keep where q >= k  <=>  col >= row
    tri01 = np.asarray(
        mask[0, 0, :c.KC, :c.KC]).T.astype(ml_dtypes.bfloat16)
    return {
        "xT": np.ascontiguousarray(xb.T.reshape(c.DC, 128, c.NT)).astype(
            ml_dtypes.bfloat16),
        "wqk": wqk_t,
        "wv": wv_t,
        "wout": wo,
        "tri": tri01,
    }


_CACHE = {}


def run_sharded(x, mask, w_qkv, w_out, trace=False, trace_cores=None):
    """Shard inputs over 8 cores, run the bass kernel, gather full output.
    Returns (out [B,N,D] f32, BassKernelResults)."""
    # the axon PJRT backend is required for execution; guard against a
    # caller environment that overrode JAX_PLATFORMS before jax init
    if 'jax' not in sys.modules and 'axon' not in os.environ.get(
            'JAX_PLATFORMS', 'axon'):
        os.environ['JAX_PLATFORMS'] = 'axon'
    from concourse.bass_utils import run_bass_kernel_spmd

    cfg = Cfg()
    B = x.shape[0]
    n_cores = 2 * B
    if 'nc' not in _CACHE:
        _CACHE['nc'] = build(cfg)
    nc = _CACHE['nc']

    x = np.asarray(x, np.float32)
    mask = np.asarray(mask)
    w_qkv = np.asarray(w_qkv, np.float32)
    w_out = np.asarray(w_out, np.float32)

    in_maps = []
    for core in range(n_cores):
        b, half = core // 2, core % 2
        in_maps.append(make_core_inputs(x[b], w_qkv, w_out, mask, cfg, half))

    if trace:
        _install_ntff_hook()
    res = run_bass_kernel_spmd(
        nc, in_maps, core_ids=list(range(n_cores)), trace=trace,
        trace_cores=trace_cores)
    outs = []
    for b in range(B):
        outs.append(res.results[2 * b]["out"].astype(np.float32)
                    + res.results[2 * b + 1]["out"].astype(np.float32))
    return np.stack(outs).astype(np.float32), res


def kernel(x, mask, w_qkv, w_out):
    out, _ = run_sharded(x, mask, w_qkv, w_out, trace=False)
    return out


# revision 25
# speedup vs baseline: 1.0085x; 1.0085x over previous
"""Self-contained TRN2 Bass kernel for nn_Attention (B=4, N=2048, D=1024, H=16).

Sharding: 8 NeuronCores, core c = (batch b = c//2, head-half = c%2).
Each core computes causal attention for its batch and 8 of 16 heads plus the
row-parallel half of the output projection; the host sums the two half-partials
per batch.

Per-core pipeline (all on-device, Tile-scheduled):
  xT [D, NT] f32  --qk-proj (bf16 matmuls)-->  qT/kT [128 feats, NT] bf16
                  --v-proj--->                 V [token, feat] bf16 (+ones col)
  S^T chunk [128 k-tok, 512 q-tok] = kT-part @ qT   (bf16, f32 psum)
  exp on ScalarE (pure Exp — no act-table thrash); causal triangle mask
  applied post-exp as a 0/1 multiply on DVE (keeps the PE free of mask work)
  O^T + softmax denominator via P^T @ [V | 1]
  den row -> DRAM-bounce partition broadcast -> fast approx reciprocal on
  DVE (partition base 0) -> normalize multiply on the idle GpSimd engine;
  the final q-chunk instead broadcasts den with a K=1 PE matmul so the
  tail projection is not gated on DMA latency
  partial out [NT, E] = OT.T @ woutT  (bf16 matmuls, f32 psum, bf16 store)
"""

import os
import sys
import types
from contextlib import ExitStack
from dataclasses import dataclass

for _p in ('/opt/trn_rl_repo', '/root/.axon_site/_ro/trn_rl_repo'):
    if os.path.isdir(_p) and _p not in sys.path:
        sys.path.append(_p)

import numpy as np
import ml_dtypes

import concourse.bass as bass
import concourse.mybir as mybir
import concourse.tile as tile
from concourse import bacc

F32 = mybir.dt.float32
BF16 = mybir.dt.bfloat16


# ---------------------------------------------------------------- harness fixes
def _install_ntff_hook():
    """Register the axon NTFF profile hook that trn_boot skips when the
    container's antenv stub lacks axon_hooks (needed only for trace=True)."""
    if 'antenv.axon_hooks' in sys.modules:
        return
    try:
        import antenv
        mod = types.ModuleType('antenv.axon_hooks')
        _hook = [None]
        mod.set_axon_ntff_profile_hook = lambda h: _hook.__setitem__(0, h)
        mod.get_axon_ntff_profile_hook = lambda: _hook[0]
        sys.modules['antenv.axon_hooks'] = mod
        antenv.axon_hooks = mod
        from trn_agent_boot.trn_boot import _ntff_profile_via_ctypes
        so = '/opt/axon/libaxon_pjrt.so'
        if os.path.exists(so):
            hook = _ntff_profile_via_ctypes(so)
            if hook is not None:
                mod.set_axon_ntff_profile_hook(hook)
    except Exception:
        pass


def _patch_tile_drain():
    """walrus TPB_CTRL encodes <=2 sync waits; Tile's tail drain can carry
    more. Split extras onto single-wait nops (sequentially equivalent)."""
    import concourse.tile as tile_mod
    if getattr(tile_mod.TileContext, '_drain_patched', False):
        return
    from concourse.tile import ScopedClock

    def _drain_and_barrier(self, tick_clock, wait_clock):
        nc = self.nc
        drain_inst = nc.sync.drain()
        wait_clock.add_sem_waits(
            drain_inst.ins, ScopedClock({None: tick_clock.global_clock}))
        si = drain_inst.ins.sync_info
        if si is not None and si.on_wait and len(si.on_wait) > 1:
            waits = list(si.on_wait)
            drain_inst.ins.sync_info = mybir.SyncInfo(
                on_wait=waits[:1], on_update=list(si.on_update or []))
            for w in waits[1:]:
                nop = nc.sync.nop(nofuse=True)
                nop.ins.sync_info = mybir.SyncInfo(on_wait=[w], on_update=[])
        nc.all_engine_barrier()
        popped = nc._tile_sem_poison_stack.pop()
        assert popped is self._sem_poison
        nc.clear_and_free_semaphores(list(self.sems.allocated().values()))
        nc.all_engine_barrier()

    tile_mod.TileContext._drain_and_barrier = _drain_and_barrier
    tile_mod.TileContext._drain_patched = True


# ---------------------------------------------------------------- kernel build
@dataclass(frozen=True)
class Cfg:
    NT: int = 2048   # tokens
    D: int = 1024    # model dim
    HH: int = 8      # heads per core
    DH: int = 64     # head dim
    E: int = 1024    # output features
    QC: int = 512    # q-chunk (free dim of score tiles)
    KC: int = 128    # k-tile (partition dim of score tiles)
    PT_BUFS: int = 8
    PSS_BUFS: int = 2
    PSO_BUFS: int = 2
    OCP_BUFS: int = 6
    DEN_BUFS: int = 6
    PSA_BUFS: int = 4

    @property
    def DC(self): return self.D // 128
    @property
    def NKT(self): return self.NT // self.KC
    @property
    def NQC(self): return self.NT // self.QC
    @property
    def NPAIR(self): return self.HH // 2
    @property
    def QF(self): return self.HH * self.DH
    @property
    def VW(self): return self.DH + 1


def build(cfg: Cfg) -> bass.Bass:
    _patch_tile_drain()
    nc = bacc.Bacc('TRN2', target_bir_lowering=False)
    c = cfg
    assert c.QC % c.KC == 0 and c.NT % c.QC == 0 and c.D % 128 == 0
    assert c.DH * 2 == c.KC
    JPT = c.QC // c.KC
    NTT = c.QC // 128

    xT = nc.declare_dram_parameter("xT", [c.DC, 128, c.NT], BF16, isOutput=False)
    wqk = nc.declare_dram_parameter("wqk", [c.DC, 128, 2 * c.QF], BF16, isOutput=False)
    wv = nc.declare_dram_parameter("wv", [c.DC, 128, c.QF], BF16, isOutput=False)
    wout = nc.declare_dram_parameter("wout", [128, c.NPAIR, c.E], BF16, isOutput=False)
    tri = nc.declare_dram_parameter("tri", [128, c.KC], BF16, isOutput=False)
    out = nc.declare_dram_parameter("out", [c.NT, c.E], BF16, isOutput=True)

    with tile.TileContext(nc) as tc, ExitStack() as ctx:
        const = ctx.enter_context(tc.tile_pool(name="const", bufs=1))
        persist = ctx.enter_context(tc.tile_pool(name="persist", bufs=1))

        # ---------------- startup-ordered DMAs ----------------
        # critical prefix: tri + wv + xT quarter 0 (feeds phase A), then wqk
        # (feeds qk prologue), then xT q1, wout, xT q2-3.
        tri_sb = const.tile([128, c.KC], BF16)
        nc.sync.dma_start(out=tri_sb[:], in_=tri[:])
        ones_bf = const.tile([128, c.DH], BF16)
        nc.vector.memset(ones_bf[:], 1.0)
        wv_sb = const.tile([128, c.DC, c.QF], BF16)
        for dc in range(c.DC):
            nc.sync.dma_start(out=wv_sb[:, dc, :], in_=wv[dc])

        xt_t = [persist.tile([128, c.NT], BF16, tag=f"xt{dc}", name=f"xt{dc}")
                for dc in range(c.DC)]
        q0 = slice(0, c.QC)
        for dc in range(c.DC):
            nc.gpsimd.dma_start(out=xt_t[dc][:, q0], in_=xT[dc][:, q0])

        wqk_sb = const.tile([128, c.DC, 2 * c.QF], BF16)
        for dc in range(c.DC):
            nc.sync.dma_start(out=wqk_sb[:, dc, :], in_=wqk[dc])

        q1 = slice(c.QC, 2 * c.QC)
        for dc in range(c.DC):
            nc.gpsimd.dma_start(out=xt_t[dc][:, q1], in_=xT[dc][:, q1])

        wout_sb = const.tile([128, c.NPAIR, c.E], BF16)
        nc.sync.dma_start(out=wout_sb[:], in_=wout[:])

        qrest = slice(2 * c.QC, c.NT)
        for dc in range(c.DC):
            nc.gpsimd.dma_start(out=xt_t[dc][:, qrest], in_=xT[dc][:, qrest])

        qk_sb = [persist.tile([128, c.NT], BF16, tag=f"qk{e}", name=f"qk{e}")
                 for e in range(2 * c.NPAIR)]
        V_sb = persist.tile([128, c.NKT, c.HH, c.VW], BF16, tag="V", name="V_sb")
        nc.vector.memset(V_sb[:, :, :, c.DH], 1.0)
        OT_sb = [persist.tile([128, c.NT], BF16, tag=f"ot{p}", name=f"ot{p}")
                 for p in range(c.NPAIR)]

        # ---------------- Phase A: v-projection, first q-chunk ----------------
        with tc.tile_pool(name="ps_a", bufs=c.PSA_BUFS, space="PSUM") as ps_a:
            for nt in range(min(NTT, c.NKT)):
                psv = ps_a.tile([128, c.QF], F32, tag="ps", name="psv")
                for dc in range(c.DC):
                    nc.tensor.matmul(
                        psv[:],
                        lhsT=xt_t[dc][:, nt * 128:(nt + 1) * 128],
                        rhs=wv_sb[:, dc, :],
                        start=(dc == 0), stop=(dc == c.DC - 1),
                    )
                nc.scalar.copy(
                    out=V_sb[:, nt, :, 0:c.DH],
                    in_=psv[:].rearrange("p (h f) -> p h f", h=c.HH),
                )

        # ---------------- Phase B: attention ----------------
        with (
            tc.tile_pool(name="pt", bufs=c.PT_BUFS) as pt_pool,
            tc.tile_pool(name="ocp", bufs=c.OCP_BUFS) as ocp_pool,
            tc.tile_pool(name="den", bufs=c.DEN_BUFS) as den_pool,
            tc.tile_pool(name="otst", bufs=4) as otst_pool,
            tc.tile_pool(name="dend", bufs=4, space="DRAM") as dend_pool,
            tc.tile_pool(name="ps_s", bufs=c.PSS_BUFS, space="PSUM") as ps_s,
            tc.tile_pool(name="ps_o", bufs=c.PSO_BUFS, space="PSUM") as ps_o,
            tc.tile_pool(name="ps_f", bufs=2, space="PSUM") as ps_f,
            tc.tile_pool(name="osbB", bufs=3) as outB_pool,
        ):
            def evac_stage1(psO):
                # free the psum banks fast: one copy per head (O rows + den)
                ocps = []
                for h2 in range(2):
                    ocp = ocp_pool.tile([c.VW, c.QC], F32, tag="ocp",
                                        name="ocp")
                    nc.vector.tensor_copy(out=ocp[:], in_=psO[h2][:])
                    ocps.append(ocp)
                return ocps

            def evac_stage2(p, t, ocps):
                # latency-tolerant: den row -> DRAM bounce partition
                # broadcast -> fast DVE reciprocal (needs partition base 0)
                # -> normalize multiply on the otherwise-idle Pool engine
                qsl_full = slice(t * c.QC, (t + 1) * c.QC)
                for h2 in range(2):
                    ocp = ocps[h2]
                    den_d = dend_pool.tile([c.QC], F32, tag="dend",
                                           name="den_d")
                    nc.sync.dma_start(out=den_d[:], in_=ocp[c.DH:c.VW, :])
                    divB = den_pool.tile([c.DH, c.QC], F32, tag="div",
                                         name="divB", bufs=4)
                    nc.sync.dma_start(
                        out=divB[:], in_=den_d.partition_broadcast(c.DH))
                    rdiv = den_pool.tile([c.DH, c.QC], F32, tag="rdiv",
                                         name="rdiv", bufs=4)
                    nc.vector.reciprocal_approx_fast(
                        out=rdiv[:], in_=divB[:])
                    if h2 == 0:
                        nc.gpsimd.tensor_tensor(
                            out=OT_sb[p][0:c.DH, qsl_full],
                            in0=ocp[0:c.DH, :],
                            in1=rdiv[:], op=mybir.AluOpType.mult,
                        )
                    else:
                        ot_st = otst_pool.tile([c.DH, c.QC], BF16, tag="ot",
                                               name="ot_st")
                        nc.gpsimd.tensor_tensor(
                            out=ot_st[:], in0=ocp[0:c.DH, :], in1=rdiv[:],
                            op=mybir.AluOpType.mult,
                        )
                        nc.sync.dma_start(
                            out=OT_sb[p][c.DH:2 * c.DH, qsl_full],
                            in_=ot_st[:])

            def emit_proj_chunk(nt, ec):
                esl = slice(ec * c.QC, (ec + 1) * c.QC)
                psP = ps_f.tile([128, c.QC], F32, tag="f", name="psP")
                for pr in range(c.NPAIR):
                    nc.tensor.matmul(
                        psP[:],
                        lhsT=OT_sb[pr][:, nt * 128:(nt + 1) * 128],
                        rhs=wout_sb[:, pr, esl],
                        start=(pr == 0), stop=(pr == c.NPAIR - 1),
                    )
                o_sb = outB_pool.tile([128, c.QC], BF16, tag="ob", name="o_sb")
                nc.vector.tensor_copy(out=o_sb[:], in_=psP[:])
                nc.sync.dma_start(
                    out=out[nt * 128:(nt + 1) * 128, esl], in_=o_sb[:])

            def emit_qk_chunk(pp, ci):
                e = (pp, c.NPAIR + pp)[ci // c.NQC]
                ncc = ci % c.NQC
                nsl = slice(ncc * c.QC, (ncc + 1) * c.QC)
                psqk = ps_f.tile([128, c.QC], F32, tag="f", name="psqk")
                for dc in range(c.DC):
                    nc.tensor.matmul(
                        psqk[:],
                        lhsT=wqk_sb[:, dc, e * 128:(e + 1) * 128],
                        rhs=xt_t[dc][:, nsl],
                        start=(dc == 0), stop=(dc == c.DC - 1),
                    )
                nc.vector.tensor_copy(out=qk_sb[e][:, nsl], in_=psqk[:])

            def emit_v_chunk(nt):
                psv = ps_f.tile([128, c.QF], F32, tag="f", name="psvf")
                for dc in range(c.DC):
                    nc.tensor.matmul(
                        psv[:],
                        lhsT=xt_t[dc][:, nt * 128:(nt + 1) * 128],
                        rhs=wv_sb[:, dc, :],
                        start=(dc == 0), stop=(dc == c.DC - 1),
                    )
                nc.scalar.copy(
                    out=V_sb[:, nt, :, 0:c.DH],
                    in_=psv[:].rearrange("p (h f) -> p h f", h=c.HH),
                )

            pending2 = None
            # prologue: only the first q-chunk's columns of q and k
            emit_qk_chunk(0, 0)
            emit_qk_chunk(0, c.NQC)
            for p in range(c.NPAIR):
                q_t, k_t = qk_sb[p], qk_sb[c.NPAIR + p]
                last_pair = (p == c.NPAIR - 1)
                for t in range(c.NQC):
                    njt = JPT * t + JPT
                    psO = [ps_o.tile([c.VW, c.QC], F32, tag="o", name=f"psO{_h}")
                           for _h in range(2)]
                    def emit_pv(items):
                        for (h2_, pt_, lo_, j_) in items:
                            nc.tensor.matmul(
                                psO[h2_][:, lo_:],
                                lhsT=V_sb[:, j_, 2 * p + h2_, :],
                                rhs=pt_[:, h2_, lo_:],
                                start=(j_ == 0), stop=(j_ == njt - 1),
                            )

                    # filler work (future qk/v projections, or output-
                    # projection chunks during the last pair), interleaved
                    # INSIDE the j-loop: the attention stream alone is paced
                    # by ScalarE's exps, so the PE needs independent matmuls
                    # queued between PV steps to stay busy
                    if p == 0:
                        fillers = []
                        if t + 1 < c.NQC:
                            # next q-chunk's V rows and q/k columns
                            fillers += [
                                lambda nt_=nt_: emit_v_chunk(nt_)
                                for nt_ in range((t + 1) * NTT,
                                                 min((t + 2) * NTT, c.NKT))]
                            fillers += [
                                lambda ci=ci: emit_qk_chunk(0, ci)
                                for ci in (t + 1, c.NQC + t + 1)]
                        fillers += [lambda i=i: emit_qk_chunk(1, 2 * t + i)
                                    for i in range(2)]
                    elif not last_pair:
                        fillers = [lambda i=i: emit_qk_chunk(p + 1, 2 * t + i)
                                   for i in range(2)]
                    elif t >= 1:
                        tp = t - 1
                        fillers = [
                            lambda nt_=nt_, ec_=ec_: emit_proj_chunk(nt_, ec_)
                            for nt_ in range(tp * NTT, (tp + 1) * NTT)
                            for ec_ in range(c.E // c.QC)]
                    else:
                        fillers = []
                    # last pair's proj fillers need OT[p] from the previous
                    # chunk's stage2 Pool chain (~4us) — hold them back a few
                    # j-steps; other fillers can start almost immediately
                    fill_start = 6 if last_pair else 1
                    spread = {}
                    if fillers and njt > fill_start:
                        span = njt - fill_start
                        nf = len(fillers)
                        for i, f in enumerate(fillers):
                            jj = fill_start + (i * span) // nf
                            spread.setdefault(jj, []).append(f)
                        fillers = []

                    pipe = []
                    for j in range(njt):
                        off = j * c.KC - t * c.QC
                        band = off >= 0
                        lo = max(off, 0)
                        jsl = slice(j * c.KC, (j + 1) * c.KC)
                        qsl = slice(t * c.QC + lo, (t + 1) * c.QC)
                        # both heads' scores into one 2-bank psum tile
                        psS = ps_s.tile([128, 2, c.QC], F32, tag="s", name="psS")
                        for h2 in range(2):
                            hsl = slice(h2 * c.DH, (h2 + 1) * c.DH)
                            nc.tensor.matmul(
                                psS[:, h2, lo:], lhsT=k_t[hsl, jsl],
                                rhs=q_t[hsl, qsl], start=True, stop=True,
                            )
                        pt_t = pt_pool.tile([128, 2, c.QC], BF16, tag="pt",
                                            name="pt_t")
                        nc.scalar.activation(
                            out=pt_t[:, :, lo:], in_=psS[:, :, lo:],
                            func=mybir.ActivationFunctionType.Exp,
                        )
                        if band:
                            # causal triangle: zero the upper-left of the
                            # diagonal block post-exp (0/1 multiply on DVE)
                            for h2 in range(2):
                                nc.vector.tensor_tensor(
                                    out=pt_t[:, h2, off:off + c.KC],
                                    in0=pt_t[:, h2, off:off + c.KC],
                                    in1=tri_sb[:],
                                    op=mybir.AluOpType.mult,
                                )
                        pipe.append([(0, pt_t, lo, j), (1, pt_t, lo, j)])
                        if len(pipe) > 4:
                            emit_pv(pipe.pop(0))
                        for f in spread.pop(j, []):
                            f()
                    # drain the PV pipe (interleaved with any leftover
                    # fillers), then evacuate psO IMMEDIATELY — before the
                    # remaining fillers' DVE evacs — so the psum banks
                    # recycle in time for the next chunk's first PV
                    while pipe:
                        if fillers:
                            fillers.pop(0)()
                        emit_pv(pipe.pop(0))
                    stage1_done = False
                    if not (last_pair and t == c.NQC - 1):
                        ocps = evac_stage1(psO)
                        stage1_done = True
                    while fillers:
                        fillers.pop(0)()
                    if last_pair and t == c.NQC - 1:
                        # fast tail: den partition-broadcast via a K=1 PE
                        # matmul (no DRAM bounce latency), recip + normalize
                        # on DVE straight from psum, then the final projs
                        qsl_full = slice(t * c.QC, (t + 1) * c.QC)
                        den_bf = den_pool.tile([128, 2, c.QC], BF16,
                                               tag="denb", name="den_bf")
                        for h2 in range(2):
                            nc.vector.tensor_copy(
                                out=den_bf[c.DH:c.VW, h2, :],
                                in_=psO[h2][c.DH:c.VW, :])
                        rdivs = []
                        for h2 in range(2):
                            psB = ps_f.tile([c.DH, c.QC], F32, tag="f",
                                            name="psB")
                            nc.tensor.matmul(
                                psB[:], lhsT=ones_bf[c.DH:c.VW, :],
                                rhs=den_bf[c.DH:c.VW, h2, :],
                                start=True, stop=True,
                            )
                            rdiv = den_pool.tile([c.DH, c.QC], F32,
                                                 tag="rdiv", name="rdiv",
                                                 bufs=4)
                            nc.vector.reciprocal_approx_fast(
                                out=rdiv[:], in_=psB[:])
                            rdivs.append(rdiv)
                        ot_st = otst_pool.tile([c.DH, c.QC], BF16, tag="ot",
                                               name="ot_st")
                        nc.vector.tensor_tensor(
                            out=ot_st[:], in0=psO[1][0:c.DH, :],
                            in1=rdivs[1][:], op=mybir.AluOpType.mult,
                        )
                        nc.sync.dma_start(
                            out=OT_sb[p][c.DH:2 * c.DH, qsl_full],
                            in_=ot_st[:])
                        nc.vector.tensor_tensor(
                            out=OT_sb[p][0:c.DH, qsl_full],
                            in0=psO[0][0:c.DH, :],
                            in1=rdivs[0][:], op=mybir.AluOpType.mult,
                        )
                        # final q-chunk projection
                        for nt_ in range(t * NTT, (t + 1) * NTT):
                            for ec_ in range(c.E // c.QC):
                                emit_proj_chunk(nt_, ec_)
                    else:
                        assert stage1_done
                        if pending2 is not None:
                            evac_stage2(*pending2)
                            pending2 = None
                        if last_pair:
                            # immediate: next chunk's proj needs OT[p=3]
                            evac_stage2(p, t, ocps)
                        else:
                            pending2 = (p, t, ocps)

    nc.compile()
    return nc


# ---------------------------------------------------------------- host side
def make_core_inputs(xb, w_qkv, w_out, mask, cfg, half):
    c = cfg
    D = c.D
    scale = 1.0 / np.sqrt(c.DH)
    heads = range(half * c.HH, (half + 1) * c.HH)
    q_rows = np.concatenate(
        [w_qkv[h * c.DH:(h + 1) * c.DH, :] for h in heads]) * scale
    k_rows = np.concatenate(
        [w_qkv[D + h * c.DH:D + (h + 1) * c.DH, :] for h in heads])
    v_rows = np.concatenate(
        [w_qkv[2 * D + h * c.DH:2 * D + (h + 1) * c.DH, :] for h in heads])
    wqk_t = np.concatenate([q_rows, k_rows], axis=0).T  # [D, 2QF]
    wqk_t = np.ascontiguousarray(
        wqk_t.reshape(c.DC, 128, 2 * c.QF)).astype(ml_dtypes.bfloat16)
    wv_t = np.ascontiguousarray(
        v_rows.T.reshape(c.DC, 128, c.QF)).astype(ml_dtypes.bfloat16)
    wo = w_out[:, half * c.QF:(half + 1) * c.QF].T  # [QF, E]
    wo = np.ascontiguousarray(
        wo.reshape(c.NPAIR, 128, c.E).transpose(1, 0, 2)).astype(
            ml_dtypes.bfloat16)
    # 0/1 causal keep-mask for the diagonal block, [k-local, q-local]:
    # keep where q >= k  <=>  col >= row
    tri01 = np.asarray(
        mask[0, 0, :c.KC, :c.KC]).T.astype(ml_dtypes.bfloat16)
    return {
        "xT": np.ascontiguousarray(xb.T.reshape(c.DC, 128, c.NT)).astype(
            ml_dtypes.bfloat16),
        "wqk": wqk_t,
        "wv": wv_t,
        "wout": wo,
        "tri": tri01,
    }


_CACHE = {}


def run_sharded(x, mask, w_qkv, w_out, trace=False, trace_cores=None):
    """Shard inputs over 8 cores, run the bass kernel, gather full output.
    Returns (out [B,N,D] f32, BassKernelResults)."""
    # the axon PJRT backend is required for execution; guard against a
    # caller environment that overrode JAX_PLATFORMS before jax init
    if 'jax' not in sys.modules and 'axon' not in os.environ.get(
            'JAX_PLATFORMS', 'axon'):
        os.environ['JAX_PLATFORMS'] = 'axon'
    from concourse.bass_utils import run_bass_kernel_spmd

    cfg = Cfg()
    B = x.shape[0]
    n_cores = 2 * B
    if 'nc' not in _CACHE:
        _CACHE['nc'] = build(cfg)
    nc = _CACHE['nc']

    x = np.asarray(x, np.float32)
    mask = np.asarray(mask)
    w_qkv = np.asarray(w_qkv, np.float32)
    w_out = np.asarray(w_out, np.float32)

    in_maps = []
    for core in range(n_cores):
        b, half = core // 2, core % 2
        in_maps.append(make_core_inputs(x[b], w_qkv, w_out, mask, cfg, half))

    if trace:
        _install_ntff_hook()
    res = run_bass_kernel_spmd(
        nc, in_maps, core_ids=list(range(n_cores)), trace=trace,
        trace_cores=trace_cores)
    outs = []
    for b in range(B):
        outs.append(res.results[2 * b]["out"].astype(np.float32)
                    + res.results[2 * b + 1]["out"].astype(np.float32))
    return np.stack(outs).astype(np.float32), res


def kernel(x, mask, w_qkv, w_out):
    out, _ = run_sharded(x, mask, w_qkv, w_out, trace=False)
    return out
